# revision 7
# baseline (speedup 1.0000x reference)
"""Multi-head causal attention (RoPE) Trainium2 kernel v3, SPMD over 8 cores.

Sharding: core = (batch b, head-group g), 4 heads each, no collectives.
Per-core schedule is a single interleaved PE stream (v2 lineage): attention
for head h hides ACT-exp latency under projection matmuls of later heads via
a virtual-clock pacing merger.

v3 changes (trace-driven):
 - tail: per-(h,c) PV outputs staged in one [128,512] SBUF tile and written
   with ONE batched out-DMA (16 instead of 64 issues; the 592ns/issue
   DIRECT2D serialization on one ring was a 13us tail after the last MM).
   Out-DMA rings rotate sync/gpsimd/scalar so the last groups issue in
   parallel.
 - PV evac moved off the DVE onto the idle ACT engine: bias is folded into
   V (out = sum a*(v+bv) / sum a == out+bv), so evac is a single
   activation(Identity, scale=1/den-vector).  Frees the DVE FIFO whose
   backlog (RoPE muls behind SWDGE swap DMAs) stalled PV psum-bank reuse.
 - PV psum banks: 2 -> 4 after head 0 (reuse the V-projection banks, freed
   once head 0's attention is done) - removes the pob bank-reuse stall.
 - startup: x streamed as 32 half-tiles round-robin across all three DGE
   rings (subtile deps let matmuls start on a half); head-0 weights are 2
   batched rearranged loads per side; cos/ssin load in halves mid-stream;
   dummies cut 48 -> 8 (HAM warm + ~2.6us bridge only).
 - head-3 RoPE swap DMAs on the scalar HWDGE ring (SWDGE latency was on the
   critical path of the g3 tail region).
 - virtual-clock constants retuned to trace-measured values.
"""

import sys

import numpy as np
import ml_dtypes

for _p in ("/opt/trn_rl_repo",):
    if _p not in sys.path:
        sys.path.insert(0, _p)

B, S, E = 2, 2048, 2048
H, D = 16, 128
P = 128
HPC = 4            # heads per core
F = HPC * D        # 512 projection features per core
NCORES = 8
NE = E // P        # 16 contraction tiles
NSQ = S // P       # 16 seq row-tiles
NCH = S // 512     # 4 chunks of 512
ROPE_BASE = 10000.0
SM_SCALE = 1.0 / float(np.sqrt(D))
BF16 = ml_dtypes.bfloat16

_compiled = None
LAST_RESULT = None

# interleaved (0,1),(2,3).. pairs -> half layout (i, i+64)
_PERM = np.concatenate([np.arange(0, D, 2), np.arange(1, D, 2)])

# virtual-clock cost estimates (ns), retuned to v2 trace
MM512 = 216.0
MMPV = 60.0
IDENT = 690.0
SWAP_LAT = 2000.0
VEVAC = 1150.0
PV_DVE = 100.0
PV_ACT = 300.0
MASK_DVE = 150.0
MARGIN = 400.0
NDUM = 12


def _mm_cost(w):
    return w / 2.4 + 6.0


def _exp_cost(w):
    return 180.0 + 0.95 * w


def _rope_tables():
    inv = ROPE_BASE ** (-np.arange(0, D, 2, dtype=np.float64) / D)
    ang = np.arange(S, dtype=np.float64)[None, :] * inv[:, None]
    cos, sin = np.cos(ang), np.sin(ang)
    cosf = np.concatenate([cos, cos], axis=0).astype(BF16)
    ssin = np.concatenate([-sin, sin], axis=0).astype(BF16)
    return cosf, ssin


def _mask_tile():
    # [128,128] lower-triangle-inclusive: mask[p, f] = 1 iff f >= p
    f = np.arange(P)[None, :]
    p = np.arange(P)[:, None]
    return (f >= p).astype(np.float32).astype(BF16)


def _build():
    import concourse.mybir as mybir
    import concourse.tile as tile
    from concourse import bacc

    fdt = mybir.dt.float32
    bdt = mybir.dt.bfloat16
    Exp = mybir.ActivationFunctionType.Exp
    Ident = mybir.ActivationFunctionType.Identity

    nc = bacc.Bacc("TRN2", target_bir_lowering=False, debug=False,
                   num_devices=NCORES)

    xt = nc.dram_tensor("xt", [E, S], bdt, kind="ExternalInput").ap()
    wqt = nc.dram_tensor("wqt", [E, F], bdt, kind="ExternalInput").ap()
    wkt = nc.dram_tensor("wkt", [E, F], bdt, kind="ExternalInput").ap()
    wvt = nc.dram_tensor("wvt", [E, F], bdt, kind="ExternalInput").ap()
    bqd = nc.dram_tensor("bqd", [P, HPC], fdt, kind="ExternalInput").ap()
    bkd = nc.dram_tensor("bkd", [P, HPC], fdt, kind="ExternalInput").ap()
    bvbd = nc.dram_tensor("bvbd", [P, F], fdt, kind="ExternalInput").ap()
    cosd = nc.dram_tensor("cosd", [P, S], bdt, kind="ExternalInput").ap()
    ssind = nc.dram_tensor("ssind", [P, S], bdt, kind="ExternalInput").ap()
    maskd = nc.dram_tensor("maskd", [P, P], bdt, kind="ExternalInput").ap()
    outd = nc.dram_tensor("out", [S, F], fdt, kind="ExternalOutput").ap()

    with tile.TileContext(nc) as tc:
        with (
            tc.tile_pool(name="const", bufs=1) as constp,
            tc.tile_pool(name="xp", bufs=1) as xp,
            tc.tile_pool(name="wvp", bufs=1) as wvp,
            tc.tile_pool(name="wqk", bufs=2) as wqk,
            tc.tile_pool(name="qk", bufs=2) as qkp,
            tc.tile_pool(name="va", bufs=1) as vap,
            tc.tile_pool(name="evac", bufs=5) as ep,
            tc.tile_pool(name="et2", bufs=2) as etp,
            tc.tile_pool(name="et3", bufs=1) as et3p,
            tc.tile_pool(name="etd", bufs=1) as etdp,
            tc.tile_pool(name="etd2", bufs=1) as etdp2,
            tc.tile_pool(name="ost", bufs=6) as osp,
        ):
            # HAM warmup source: dependency-free dummy matmuls bridge the
            # startup DMA wait so the clock gate is at 8/8 for real work
            dum = constp.tile([P, 256], bdt, tag="dum", name="dum")
            nc.vector.memset(dum[:], 0.0)

            # ---- tiny constants first on the gpsimd (SWDGE) ring ----
            bqpt = constp.tile([P, HPC], fdt, tag="bqpt", name="bqpt")
            nc.gpsimd.dma_start(bqpt[:], bqd[:])
            bkpt = constp.tile([P, HPC], fdt, tag="bkpt", name="bkpt")
            nc.gpsimd.dma_start(bkpt[:], bkd[:])
            mask_sb = constp.tile([P, P], bdt, tag="mask", name="mask_sb")
            nc.gpsimd.dma_start(mask_sb[:], maskd[:])

            # ---- startup streaming ----
            # Everything needed in the first ~40us is HBM-BW bound (~9MB):
            # strict priority order, striped across all three DGE rings.
            wq_all = {}
            wk_all = {}

            def wload_part(dram, h, nm, ring, t, e0, e1):
                # e-tiles e0..e1 of head h into t[:, P*e0 : P*e1]
                src = dram[P * e0:P * e1, P * h:P * (h + 1)].rearrange(
                    "(e p) f -> p e f", p=P)
                ring.dma_start(
                    t[:, P * e0:P * e1].rearrange("p (e f) -> p e f",
                                                  e=e1 - e0), src)

            # head-0 weights: small leading chunk (e0-3) for the first
            # matmuls, then the rest batched
            wq_all[0] = wqk.tile([P, S], bdt, tag="wq", name="wq0")
            wk_all[0] = wqk.tile([P, S], bdt, tag="wk", name="wk0")
            wload_part(wqt, 0, "wq0a", nc.scalar, wq_all[0], 0, 4)
            wload_part(wkt, 0, "wk0a", nc.gpsimd, wk_all[0], 0, 4)

            # x: 32 half-tiles striped round-robin; extras inserted by need
            xts = [None] * NE
            for e in range(NE):
                xts[e] = xp.tile([P, S], bdt, tag=f"x{e}", name=f"x{e}")
            rings3 = (nc.sync, nc.scalar, nc.gpsimd)
            ri = 0

            def stripe(dst, src):
                nonlocal ri
                rings3[ri % 3].dma_start(dst, src)
                ri += 1

            cos_sb = constp.tile([P, S], bdt, tag="cos", name="cos_sb")
            ssin_sb = constp.tile([P, S], bdt, tag="ssin", name="ssin_sb")
            wv_all = wvp.tile([P, NE * F], bdt, tag="wv", name="wv_all")
            bvb_sb = constp.tile([P, F], fdt, tag="bvb", name="bvb_sb")

            half = S // 2
            for e in range(NE):
                for hh in range(2):
                    cs = slice(half * hh, half * (hh + 1))
                    stripe(xts[e][:, cs], xt[P * e:P * (e + 1), cs])
                if e == 2:
                    wload_part(wqt, 0, "wq0b", nc.scalar, wq_all[0], 4, NE)
                    wload_part(wkt, 0, "wk0b", nc.gpsimd, wk_all[0], 4, NE)
                if e == 11:
                    # RoPE tables for the head-0 evacs (~+28us)
                    nc.scalar.dma_start(cos_sb[:, 0:half], cosd[:, 0:half])
                    nc.gpsimd.dma_start(ssin_sb[:, 0:half], ssind[:, 0:half])
                if e == 13:
                    nc.scalar.dma_start(cos_sb[:, half:], cosd[:, half:])
                    nc.gpsimd.dma_start(ssin_sb[:, half:], ssind[:, half:])
            # wv just behind x (pad v-units start right after head-0 QK)
            for q in range(4):
                nc.scalar.dma_start(
                    wv_all[:, 4 * F * q:4 * F * (q + 1)].rearrange(
                        "p (e f) -> p e f", e=4),
                    wvt[4 * P * q:4 * P * (q + 1), :].rearrange(
                        "(e p) f -> p e f", p=P))
            nc.gpsimd.dma_start(bvb_sb[:], bvbd[:])

            def load_wqk(h):
                wq_all[h] = wqk.tile([P, S], bdt, tag="wq", name=f"wq{h}")
                wk_all[h] = wqk.tile([P, S], bdt, tag="wk", name=f"wk{h}")
                wload_part(wqt, h, f"wq{h}", nc.sync, wq_all[h], 0, NE)
                wload_part(wkt, h, f"wk{h}", nc.scalar, wk_all[h], 0, NE)

            qT = {}
            kT = {}
            vA = [[vap.tile([P, D + 1], bdt, tag=f"vA{h}_{j}",
                            name=f"vA{h}_{j}")
                   for j in range(NSQ)] for h in range(HPC)]
            for h in range(HPC):
                for j in range(NSQ):
                    nc.vector.memset(vA[h][j][:, D:D + 1], 1.0)

            # virtual clocks (ns)
            clk = {"pe": 0.0, "act": 0.0, "dve": 0.0}

            def on_pe(cost):
                clk["pe"] += cost

            def on_act(cost):
                clk["act"] = max(clk["act"], clk["pe"]) + cost

            def on_dve(cost, ready=None):
                base = max(clk["dve"], clk["pe"] if ready is None else ready)
                clk["dve"] = base + cost

            # ---- evac + RoPE chain for a finished QK psum bank ----
            # swap rings: h0 sync (latency-critical at start), h3 scalar
            # (latency-critical in the g3 tail), h1/h2 on slow SWDGE.
            def qk_evac(ps_t, h, c, is_q):
                cs = slice(512 * c, 512 * (c + 1))
                dst = qT[h] if is_q else kT[h]
                bias = bqpt if is_q else bkpt
                nm = f"{'q' if is_q else 'k'}{h}_{c}"
                xs = ep.tile([P, 512], bdt, tag="xs", name=f"xs{nm}")
                nc.scalar.activation(xs[:], ps_t[:], Ident,
                                     bias=bias[:, h:h + 1])
                on_act(IDENT)
                sw = ep.tile([P, 512], bdt, tag="sw", name=f"sw{nm}")
                swr = (nc.sync if h == 0 else
                       nc.scalar if h == 3 else nc.gpsimd)
                swr.dma_start(sw[0:64, :], xs[64:128, :])
                swr.dma_start(sw[64:128, :], xs[0:64, :])
                t2 = ep.tile([P, 512], bdt, tag="t2", name=f"t2{nm}")
                nc.vector.tensor_mul(dst[:, cs], xs[:], cos_sb[:, cs])
                nc.vector.tensor_mul(t2[:], sw[:], ssin_sb[:, cs])
                nc.vector.tensor_add(dst[:, cs], dst[:, cs], t2[:])
                on_dve(3 * 450.0, ready=clk["act"] + SWAP_LAT)

            # ================= QK head 0: e-outer, 8 banks =================
            # All Q/K chunks accumulate e-outer (each step needs only x[e]),
            # finals stagger per bank so RoPE/scores(0,0) start early.
            with tc.tile_pool(name="ps0", bufs=1, space="PSUM") as ps0:
                qT[0] = qkp.tile([P, S], bdt, tag="qT", name="qT0")
                kT[0] = qkp.tile([P, S], bdt, tag="kT", name="kT0")
                bank = {}
                parts = [(m, c) for c in range(NCH) for m in ("q", "k")]
                for m, c in parts:
                    bank[(m, c)] = ps0.tile([P, 512], fdt, tag=f"b{m}{c}",
                                            name=f"ps{m}0_{c}")
                for i in range(NDUM):
                    nc.tensor.matmul(bank[("q", 0)][:, 0:256], dum[:, 0:P],
                                     dum[:], start=True, stop=True)
                for e in range(NE - 4):
                    for m, c in parts:
                        wt = wq_all[0] if m == "q" else wk_all[0]
                        nc.tensor.matmul(
                            bank[(m, c)][:], wt[:, P * e:P * (e + 1)],
                            xts[e][:, 512 * c:512 * (c + 1)],
                            start=(e == 0), stop=False)
                        on_pe(MM512)
                # staggered finals: finish one (m, c) bank at a time and
                # evacuate immediately
                for m, c in parts:
                    wt = wq_all[0] if m == "q" else wk_all[0]
                    for e in range(NE - 4, NE):
                        nc.tensor.matmul(
                            bank[(m, c)][:], wt[:, P * e:P * (e + 1)],
                            xts[e][:, 512 * c:512 * (c + 1)],
                            start=False, stop=(e == NE - 1))
                        on_pe(MM512)
                    qk_evac(bank[(m, c)], 0, c, m == "q")

            # ================= main pools + pad-unit machinery ============
            with tc.tile_pool(name="psm", bufs=1, space="PSUM") as psm:
                # alloc order maps tiles to the physical banks freed earliest
                vvb = [psm.tile([P, 512], fdt, tag=f"vv{i}", name=f"vv{i}")
                       for i in range(2)]
                scb = [psm.tile([P, 512], fdt, tag=f"sc{i}", name=f"sc{i}")
                       for i in range(2)]
                prj = [psm.tile([P, 512], fdt, tag=f"prj{i}", name=f"prj{i}")
                       for i in range(2)]
                pob = [psm.tile([P, D + 1], fdt, tag=f"po{i}", name=f"po{i}")
                       for i in range(2)]

                # out-DMA ring rotation for the batched per-(h,c) stores
                orings = [nc.sync, nc.gpsimd, nc.scalar]
                ocnt = [0]

                units = []          # (key, cost, fn)
                emitted = set()

                def emit_one():
                    key, cost, fn = units.pop(0)
                    fn()
                    on_pe(cost)
                    emitted.add(key)

                def pad_until(target):
                    while clk["pe"] < target and units:
                        emit_one()

                def flush_until(key):
                    while key not in emitted and units:
                        emit_one()

                # --- unit builders ---
                def v_unit(j, e0):
                    def fn(j=j, e0=e0):
                        vb = vvb[j % 2]
                        for e in range(e0, e0 + 4):
                            nc.tensor.matmul(
                                vb[:], xts[e][:, P * j:P * (j + 1)],
                                wv_all[:, F * e:F * (e + 1)],
                                start=(e == 0), stop=(e == NE - 1))
                        if e0 == 12:
                            # fold the v bias in here: out of the softmax,
                            # sum a*(v+bv)/sum a == sum a*v/sum a + bv
                            for h in range(HPC):
                                nc.vector.tensor_add(
                                    vA[h][j][:, 0:D], vb[:, D * h:D * (h + 1)],
                                    bvb_sb[:, D * h:D * (h + 1)])
                            on_dve(VEVAC)
                    return (("v", j, e0), 4 * MM512, fn)

                def qk_unit(m, h, c, e0, mk=False):
                    def fn(m=m, h=h, c=c, e0=e0, mk=mk):
                        if mk and m == "q":
                            qT[h] = qkp.tile([P, S], bdt, tag="qT",
                                             name=f"qT{h}")
                        if mk and m == "k":
                            kT[h] = qkp.tile([P, S], bdt, tag="kT",
                                             name=f"kT{h}")
                        pt = prj[0] if m == "q" else prj[1]
                        wt = wq_all[h] if m == "q" else wk_all[h]
                        for e in range(e0, e0 + 4):
                            nc.tensor.matmul(
                                pt[:], wt[:, P * e:P * (e + 1)],
                                xts[e][:, 512 * c:512 * (c + 1)],
                                start=(e == 0), stop=(e == NE - 1))
                        if e0 == 12:
                            qk_evac(pt, h, c, m == "q")
                    return ((m, h, c, e0), 4 * MM512, fn)

                def queue_v(j):
                    for e0 in (0, 4, 8, 12):
                        units.append(v_unit(j, e0))

                def queue_qk(h):
                    load_wqk(h)
                    if h == 3:
                        # K first (scores(3,c) need all K chunks <= c), Q in
                        # descending-c order matching head-3's chunk order:
                        # leftover Q units double as tail PE pad work.
                        seq = [("k", c) for c in range(NCH)]
                        seq += [("q", c) for c in (3, 2, 1, 0)]
                    else:
                        seq = [(m, c) for c in range(NCH)
                               for m in ("q", "k")]
                    seen = set()
                    for m, c in seq:
                        for e0 in (0, 4, 8, 12):
                            units.append(
                                qk_unit(m, h, c, e0,
                                        mk=(m not in seen and e0 == 0)))
                        seen.add(m)

                # --- attention ---
                def attn(h, c, defer_pv=False, pool=None):
                    pool = pool or etp
                    nt = 4 * c + 4
                    if h == 3:
                        flush_until(("q", h, c, 12))
                    elif h > 0:
                        flush_until(("k", h, c, 12))
                    order = list(range(4 * c, nt)) + list(range(0, 4 * c))
                    sel = [None] * nt     # (tile, col_offset)
                    selm = [None] * nt    # masked diag block [P,128]
                    for t in order:
                        diag = t >= 4 * c
                        o = P * (t % 4) if diag else 0
                        w = 512 - o
                        cs = slice(512 * c + o, 512 * (c + 1))
                        ps_sc = scb[t % 2]
                        nc.tensor.matmul(ps_sc[:, 0:w],
                                         kT[h][:, P * t:P * (t + 1)],
                                         qT[h][:, cs], start=True, stop=True)
                        on_pe(_mm_cost(w))
                        tp = pool if pool is etdp else (
                            et3p if t >= 12 else etp)
                        et = tp.tile([P, w], bdt, tag=f"et{t}",
                                     name=f"et{h}_{c}_{t}")
                        nc.scalar.activation(et[:], ps_sc[:, 0:w], Exp,
                                             scale=SM_SCALE)
                        on_act(_exp_cost(w))
                        sel[t] = (et, o)
                        if diag:
                            etm = tp.tile([P, P], bdt, tag=f"etm{t % 4}",
                                          name=f"etm{h}_{c}_{t}")
                            nc.vector.tensor_mul(etm[:], et[:, 0:P],
                                                 mask_sb[:])
                            on_dve(MASK_DVE)
                            selm[t] = etm
                        pad_until(clk["act"] - 2 * _exp_cost(512))
                    gate = max(clk["act"], clk["dve"])
                    if defer_pv:
                        return sel, selm
                    if h == 0:
                        flush_until(("v", 4 * c + 3, 12))
                    pad_until(gate + MARGIN)
                    pv(h, c, sel, selm)

                def pv(h, c, sel, selm, as_units=False):
                    mk = []
                    ogc = []   # group staging tile, created lazily at emit

                    def do_j(jj, h, c, sel, selm):
                        if not ogc:
                            ogc.append(osp.tile([P, 4 * D], fdt, tag="og",
                                                name=f"og{h}_{c}", bufs=3))
                        og = ogc[0]
                        j = 4 * c + jj
                        po = pob[jj % len(pob)]
                        for t in range(j + 1):
                            if t == j:
                                src = selm[t][:]
                            else:
                                et, o = sel[t]
                                lo = P * jj - o
                                src = et[:, lo:lo + P]
                            nc.tensor.matmul(po[:], src, vA[h][t][:],
                                             start=(t == 0), stop=(t == j))
                        rec = osp.tile([P, 1], fdt, tag="rec",
                                       name=f"rec{h}_{j}")
                        nc.vector.reciprocal(rec[:], po[:, D:D + 1])
                        on_dve(PV_DVE)
                        # PV evac on the ACT engine: out = po * (1/den)
                        nc.scalar.activation(og[:, D * jj:D * (jj + 1)],
                                             po[:, 0:D], Ident, scale=rec[:])
                        on_act(PV_ACT)
                        if jj == 3:
                            r = orings[ocnt[0] % 3]
                            ocnt[0] += 1
                            r.dma_start(
                                outd[512 * c:512 * (c + 1),
                                     D * h:D * (h + 1)].rearrange(
                                         "(j p) d -> p j d", p=P),
                                og[:].rearrange("p (j d) -> p j d", j=4))

                    for jj in range(4):
                        def fn(jj=jj, h=h, c=c, sel=sel, selm=selm):
                            do_j(jj, h, c, sel, selm)
                        cost = (4 * c + jj + 1) * MMPV
                        if as_units:
                            mk.append((("pvd", h, c, jj), cost, fn))
                        else:
                            fn()
                            on_pe(cost)
                    return mk

                # ---------------- the schedule ----------------
                for j in range(NSQ):
                    queue_v(j)
                queue_qk(1)
                # bridge the head-0 RoPE-chain latency with V work so the
                # PE isn't head-of-line blocked on scores(0,0)'s inputs
                flush_until(("v", 1, 12))
                for c in range(NCH):
                    attn(0, c)
                # head-0 attention done: V psum banks are free; deepen the
                # PV rotation to 4 banks for heads 1-3
                pob.append(psm.tile([P, D + 1], fdt, tag="vv0", name="po2"))
                pob.append(psm.tile([P, D + 1], fdt, tag="vv1", name="po3"))
                queue_qk(2)
                d1 = None
                for c in range(NCH):
                    if c == 3:
                        d1 = attn(1, c, defer_pv=True, pool=etdp2)
                    else:
                        attn(1, c)
                queue_qk(3)
                dsel = dselm = None
                for c in range(NCH):
                    if c == 3:
                        dsel, dselm = attn(2, c, defer_pv=True, pool=etdp)
                    else:
                        attn(2, c)
                # head 3 descending, deferred PV(1,3)+PV(2,3) as tail pads
                units.extend(pv(1, 3, d1[0], d1[1], as_units=True))
                units.extend(pv(2, 3, dsel, dselm, as_units=True))
                g3 = {}
                for c in (3, 2, 1, 0):
                    s, m = attn(3, c, defer_pv=True)
                    g3[c] = (s, m, max(clk["act"], clk["dve"]))
                    if c == 3:
                        continue
                    sd, md, gate = g3[c + 1]
                    if c == 0:
                        # spend reserve pads now: the kernel must end on
                        # dependency-free matmuls, not an exp wait
                        while len(units) > 2:
                            emit_one()
                    pad_until(gate + MARGIN)
                    pv(3, c + 1, sd, md)
                sd, md, gate = g3[0]
                while units:
                    emit_one()
                pv(3, 0, sd, md)

    nc.compile()
    return nc


def get_compiled():
    global _compiled
    if _compiled is None:
        _compiled = _build()
    return _compiled


def expected_slice_core0(expected):
    return expected[0, :, 0:F]


def make_in_maps(logits, Wq, bq, Wk, bk, Wv, bv):
    cosf, ssin = _rope_tables()
    maskm = _mask_tile()
    xts = [np.ascontiguousarray(np.asarray(logits)[b].T).astype(BF16)
           for b in range(B)]

    def permW(Wm, rows):
        Wp = np.asarray(Wm)[rows].reshape(HPC, D, E)[:, _PERM, :].reshape(F, E)
        return np.ascontiguousarray(Wp.T).astype(BF16)

    def permb(bvec, rows):
        return np.ascontiguousarray(
            np.asarray(bvec)[rows].reshape(HPC, D)[:, _PERM].T
        ).astype(np.float32)

    in_maps = []
    for core in range(NCORES):
        b, g = divmod(core, 4)
        rows = slice(F * g, F * (g + 1))
        in_maps.append({
            "xt": xts[b],
            "wqt": permW(Wq, rows),
            "wkt": permW(Wk, rows),
            "wvt": np.ascontiguousarray(np.asarray(Wv)[rows].T).astype(BF16),
            "bqd": permb(bq, rows),
            "bkd": permb(bk, rows),
            "bvbd": np.ascontiguousarray(np.broadcast_to(
                np.asarray(bv)[rows].astype(np.float32), (P, F))),
            "cosd": cosf,
            "ssind": ssin,
            "maskd": maskm,
        })
    return in_maps


def kernel(logits, Wq, bq, Wk, bk, Wv, bv, **_ignored):
    global LAST_RESULT
    from concourse.bass_utils import run_bass_kernel_spmd

    nc = get_compiled()
    in_maps = make_in_maps(logits, Wq, bq, Wk, bk, Wv, bv)
    res = run_bass_kernel_spmd(nc, in_maps, list(range(NCORES)))
    LAST_RESULT = res
    out = np.empty((B, S, H * D), dtype=np.float32)
    for core in range(NCORES):
        b, g = divmod(core, 4)
        out[b, :, F * g:F * (g + 1)] = res.results[core]["out"]
    return out


# revision 18
# speedup vs baseline: 1.0351x; 1.0351x over previous
"""Multi-head causal attention (RoPE) Trainium2 kernel v3, SPMD over 8 cores.

Sharding: core = (batch b, head-group g), 4 heads each, no collectives.
Per-core schedule is a single interleaved PE stream (v2 lineage): attention
for head h hides ACT-exp latency under projection matmuls of later heads via
a virtual-clock pacing merger.

v3 changes (trace-driven):
 - tail: per-(h,c) PV outputs staged in one [128,512] SBUF tile and written
   with ONE batched out-DMA (16 instead of 64 issues; the 592ns/issue
   DIRECT2D serialization on one ring was a 13us tail after the last MM).
   Out-DMA rings rotate sync/gpsimd/scalar so the last groups issue in
   parallel.
 - PV evac moved off the DVE onto the idle ACT engine: bias is folded into
   V (out = sum a*(v+bv) / sum a == out+bv), so evac is a single
   activation(Identity, scale=1/den-vector).  Frees the DVE FIFO whose
   backlog (RoPE muls behind SWDGE swap DMAs) stalled PV psum-bank reuse.
 - PV psum banks: 2 -> 4 after head 0 (reuse the V-projection banks, freed
   once head 0's attention is done) - removes the pob bank-reuse stall.
 - startup: x streamed as 32 half-tiles round-robin across all three DGE
   rings (subtile deps let matmuls start on a half); head-0 weights are 2
   batched rearranged loads per side; cos/ssin load in halves mid-stream;
   dummies cut 48 -> 8 (HAM warm + ~2.6us bridge only).
 - head-3 RoPE swap DMAs on the scalar HWDGE ring (SWDGE latency was on the
   critical path of the g3 tail region).
 - virtual-clock constants retuned to trace-measured values.
"""

import sys

import numpy as np
import ml_dtypes

for _p in ("/opt/trn_rl_repo",):
    if _p not in sys.path:
        sys.path.insert(0, _p)

B, S, E = 2, 2048, 2048
H, D = 16, 128
P = 128
HPC = 4            # heads per core
F = HPC * D        # 512 projection features per core
NCORES = 8
NE = E // P        # 16 contraction tiles
NSQ = S // P       # 16 seq row-tiles
NCH = S // 512     # 4 chunks of 512
ROPE_BASE = 10000.0
SM_SCALE = 1.0 / float(np.sqrt(D))
BF16 = ml_dtypes.bfloat16

_compiled = None
LAST_RESULT = None

# interleaved (0,1),(2,3).. pairs -> half layout (i, i+64)
_PERM = np.concatenate([np.arange(0, D, 2), np.arange(1, D, 2)])

# virtual-clock cost estimates (ns), retuned to v2 trace
MM512 = 216.0
MMPV = 60.0
IDENT = 690.0
SWAP_LAT = 2500.0
VEVAC = 1150.0
PV_DVE = 100.0
PV_ACT = 300.0
MASK_DVE = 150.0
MARGIN = 400.0
NDUM = 12


def _mm_cost(w):
    return w / 2.4 + 6.0


def _exp_cost(w):
    return 180.0 + 0.95 * w


def _rope_tables():
    inv = ROPE_BASE ** (-np.arange(0, D, 2, dtype=np.float64) / D)
    ang = np.arange(S, dtype=np.float64)[None, :] * inv[:, None]
    cos, sin = np.cos(ang), np.sin(ang)
    cosf = np.concatenate([cos, cos], axis=0).astype(BF16)
    ssin = np.concatenate([-sin, sin], axis=0).astype(BF16)
    return cosf, ssin


def _mask_tile():
    # [128,128] lower-triangle-inclusive: mask[p, f] = 1 iff f >= p
    f = np.arange(P)[None, :]
    p = np.arange(P)[:, None]
    return (f >= p).astype(np.float32).astype(BF16)


def _build():
    import concourse.mybir as mybir
    import concourse.tile as tile
    from concourse import bacc

    fdt = mybir.dt.float32
    bdt = mybir.dt.bfloat16
    Exp = mybir.ActivationFunctionType.Exp
    Ident = mybir.ActivationFunctionType.Identity

    nc = bacc.Bacc("TRN2", target_bir_lowering=False, debug=False,
                   num_devices=NCORES)

    xt = nc.dram_tensor("xt", [E, S], bdt, kind="ExternalInput").ap()
    # weights pre-arranged on the host into the SBUF layout so every load
    # is a plain contiguous DMA (rearranged-AP loads cost 3-8us to issue)
    wqt = nc.dram_tensor("wqt", [P, HPC * S], bdt, kind="ExternalInput").ap()
    wkt = nc.dram_tensor("wkt", [P, HPC * S], bdt, kind="ExternalInput").ap()
    wvt = nc.dram_tensor("wvt", [P, NE * F], bdt, kind="ExternalInput").ap()
    bqd = nc.dram_tensor("bqd", [P, HPC], fdt, kind="ExternalInput").ap()
    bkd = nc.dram_tensor("bkd", [P, HPC], fdt, kind="ExternalInput").ap()
    bvbd = nc.dram_tensor("bvbd", [P, F], fdt, kind="ExternalInput").ap()
    cosd = nc.dram_tensor("cosd", [P, S], bdt, kind="ExternalInput").ap()
    ssind = nc.dram_tensor("ssind", [P, S], bdt, kind="ExternalInput").ap()
    maskd = nc.dram_tensor("maskd", [P, P], bdt, kind="ExternalInput").ap()
    outd = nc.dram_tensor("out", [S, F], fdt, kind="ExternalOutput").ap()

    with tile.TileContext(nc) as tc:
        with (
            tc.tile_pool(name="const", bufs=1) as constp,
            tc.tile_pool(name="xp", bufs=1) as xp,
            tc.tile_pool(name="wvp", bufs=1) as wvp,
            tc.tile_pool(name="wqk", bufs=2) as wqk,
            tc.tile_pool(name="qk", bufs=2) as qkp,
            tc.tile_pool(name="va", bufs=1) as vap,
            tc.tile_pool(name="evac", bufs=5) as ep,
            tc.tile_pool(name="et2", bufs=2) as etp,
            tc.tile_pool(name="et3", bufs=1) as et3p,
            tc.tile_pool(name="etd", bufs=1) as etdp,
            tc.tile_pool(name="etd2", bufs=1) as etdp2,
            tc.tile_pool(name="ost", bufs=6) as osp,
        ):
            # HAM warmup source: dependency-free dummy matmuls bridge the
            # startup DMA wait so the clock gate is at 8/8 for real work
            dum = constp.tile([P, 256], bdt, tag="dum", name="dum")
            nc.vector.memset(dum[:], 0.0)

            # ---- tiny constants first on the gpsimd (SWDGE) ring ----
            bqpt = constp.tile([P, HPC], fdt, tag="bqpt", name="bqpt")
            nc.gpsimd.dma_start(bqpt[:], bqd[:])
            bkpt = constp.tile([P, HPC], fdt, tag="bkpt", name="bkpt")
            nc.gpsimd.dma_start(bkpt[:], bkd[:])
            mask_sb = constp.tile([P, P], bdt, tag="mask", name="mask_sb")
            nc.gpsimd.dma_start(mask_sb[:], maskd[:])

            # ---- startup streaming ----
            # Everything needed in the first ~40us is HBM-BW bound (~9MB):
            # strict priority order, striped across all three DGE rings.
            wq_all = {}
            wk_all = {}
            half = S // 2

            # head-0 weights in halves so e0 matmuls can start early
            wq_all[0] = wqk.tile([P, S], bdt, tag="wq", name="wq0")
            wk_all[0] = wqk.tile([P, S], bdt, tag="wk", name="wk0")

            xts = [None] * NE
            for e in range(NE):
                xts[e] = xp.tile([P, S], bdt, tag=f"x{e}", name=f"x{e}")
            cos_sb = constp.tile([P, S], bdt, tag="cos", name="cos_sb")
            ssin_sb = constp.tile([P, S], bdt, tag="ssin", name="ssin_sb")
            wv_all = wvp.tile([P, NE * F], bdt, tag="wv", name="wv_all")
            bvb_sb = constp.tile([P, F], fdt, tag="bvb", name="bvb_sb")

            # per-ring issue sequences, ordered by first-use time.  sync is
            # kept w0-free so x0/x1 land first; w0 second-halves and tables
            # slot in just ahead of their consumers; wv quarters arrive for
            # the v-unit pads right after the head-0 e-outer phase.
            def _x(e, hh):
                cs = slice(half * hh, half * (hh + 1))
                return (xts[e][:, cs], xt[P * e:P * (e + 1), cs])

            def _wq(hh):
                cs = slice(half * hh, half * (hh + 1))
                return (wq_all[0][:, cs], wqt[:, cs])

            def _wk(hh):
                cs = slice(half * hh, half * (hh + 1))
                return (wk_all[0][:, cs], wkt[:, cs])

            def _wv(q):
                cs = slice(4 * F * q, 4 * F * (q + 1))
                return (wv_all[:, cs], wvt[:, cs])

            def _tab(t, hh):
                cs = slice(half * hh, half * (hh + 1))
                return ((cos_sb[:, cs], cosd[:, cs]) if t == 0 else
                        (ssin_sb[:, cs], ssind[:, cs]))

            seq_sync = ([_x(0, 0), _x(0, 1), _x(1, 0), _x(1, 1)] +
                        [_x(e, hh) for e in (4, 7, 10, 13) for hh in (0, 1)] +
                        [_wv(0)])
            seq_scalar = ([_wq(0)] +
                          [_x(e, hh) for e in (2, 5) for hh in (0, 1)] +
                          [_wq(1)] +
                          [_x(e, hh) for e in (8, 11) for hh in (0, 1)] +
                          [_tab(0, 0)] + [_x(14, 0), _x(14, 1)] +
                          [_tab(0, 1), _wv(1), _wv(3)])
            seq_gpsimd = ([_wk(0)] +
                          [_x(e, hh) for e in (3, 6) for hh in (0, 1)] +
                          [_wk(1)] +
                          [_x(e, hh) for e in (9, 12) for hh in (0, 1)] +
                          [_tab(1, 0)] + [_x(15, 0), _x(15, 1)] +
                          [_tab(1, 1), _wv(2), (bvb_sb[:], bvbd[:])])
            for ring, seqq in ((nc.sync, seq_sync), (nc.scalar, seq_scalar),
                               (nc.gpsimd, seq_gpsimd)):
                for dst, src in seqq:
                    ring.dma_start(dst, src)

            def load_wqk(h):
                wq_all[h] = wqk.tile([P, S], bdt, tag="wq", name=f"wq{h}")
                wk_all[h] = wqk.tile([P, S], bdt, tag="wk", name=f"wk{h}")
                nc.sync.dma_start(wq_all[h][:], wqt[:, S * h:S * (h + 1)])
                nc.scalar.dma_start(wk_all[h][:], wkt[:, S * h:S * (h + 1)])

            qT = {}
            kT = {}
            vA = [[vap.tile([P, D + 1], bdt, tag=f"vA{h}_{j}",
                            name=f"vA{h}_{j}")
                   for j in range(NSQ)] for h in range(HPC)]
            for h in range(HPC):
                for j in range(NSQ):
                    nc.vector.memset(vA[h][j][:, D:D + 1], 1.0)

            # virtual clocks (ns)
            clk = {"pe": 0.0, "act": 0.0, "dve": 0.0}

            def on_pe(cost):
                clk["pe"] += cost

            def on_act(cost):
                clk["act"] = max(clk["act"], clk["pe"]) + cost

            def on_dve(cost, ready=None):
                base = max(clk["dve"], clk["pe"] if ready is None else ready)
                clk["dve"] = base + cost

            # ---- evac + RoPE chain for a finished QK psum bank ----
            # swap rings: h0 sync (latency-critical at start), h3 scalar
            # (latency-critical in the g3 tail), h1/h2 on slow SWDGE.
            def qk_evac(ps_t, h, c, is_q):
                cs = slice(512 * c, 512 * (c + 1))
                dst = qT[h] if is_q else kT[h]
                bias = bqpt if is_q else bkpt
                nm = f"{'q' if is_q else 'k'}{h}_{c}"
                xs = ep.tile([P, 512], bdt, tag="xs", name=f"xs{nm}")
                nc.scalar.activation(xs[:], ps_t[:], Ident,
                                     bias=bias[:, h:h + 1])
                on_act(IDENT)
                sw = ep.tile([P, 512], bdt, tag="sw", name=f"sw{nm}")
                swr = (nc.sync if h == 0 else
                       nc.scalar if h == 3 else nc.gpsimd)
                swr.dma_start(sw[0:64, :], xs[64:128, :])
                swr.dma_start(sw[64:128, :], xs[0:64, :])
                t2 = ep.tile([P, 512], bdt, tag="t2", name=f"t2{nm}")
                nc.vector.tensor_mul(dst[:, cs], xs[:], cos_sb[:, cs])
                nc.vector.tensor_mul(t2[:], sw[:], ssin_sb[:, cs])
                nc.vector.tensor_add(dst[:, cs], dst[:, cs], t2[:])
                on_dve(3 * 450.0, ready=clk["act"] + SWAP_LAT)

            # ================= QK head 0: e-outer, 8 banks =================
            # All Q/K chunks accumulate e-outer (each step needs only x[e]),
            # finals stagger per bank so RoPE/scores(0,0) start early.
            with tc.tile_pool(name="ps0", bufs=1, space="PSUM") as ps0:
                qT[0] = qkp.tile([P, S], bdt, tag="qT", name="qT0")
                kT[0] = qkp.tile([P, S], bdt, tag="kT", name="kT0")
                bank = {}
                parts = [(m, c) for c in range(NCH) for m in ("q", "k")]
                for m, c in parts:
                    bank[(m, c)] = ps0.tile([P, 512], fdt, tag=f"b{m}{c}",
                                            name=f"ps{m}0_{c}")
                for i in range(NDUM):
                    nc.tensor.matmul(bank[("q", 0)][:, 0:256], dum[:, 0:P],
                                     dum[:], start=True, stop=True)
                for e in range(NE - 4):
                    for m, c in parts:
                        wt = wq_all[0] if m == "q" else wk_all[0]
                        nc.tensor.matmul(
                            bank[(m, c)][:], wt[:, P * e:P * (e + 1)],
                            xts[e][:, 512 * c:512 * (c + 1)],
                            start=(e == 0), stop=False)
                        on_pe(MM512)
                # staggered finals: finish one (m, c) bank at a time and
                # evacuate immediately
                for m, c in parts:
                    wt = wq_all[0] if m == "q" else wk_all[0]
                    for e in range(NE - 4, NE):
                        nc.tensor.matmul(
                            bank[(m, c)][:], wt[:, P * e:P * (e + 1)],
                            xts[e][:, 512 * c:512 * (c + 1)],
                            start=False, stop=(e == NE - 1))
                        on_pe(MM512)
                    qk_evac(bank[(m, c)], 0, c, m == "q")

            # ================= main pools + pad-unit machinery ============
            with tc.tile_pool(name="psm", bufs=1, space="PSUM") as psm:
                # alloc order maps tiles to the physical banks freed earliest
                vvb = [psm.tile([P, 512], fdt, tag=f"vv{i}", name=f"vv{i}")
                       for i in range(2)]
                scb = [psm.tile([P, 512], fdt, tag=f"sc{i}", name=f"sc{i}")
                       for i in range(2)]
                prj = [psm.tile([P, 512], fdt, tag=f"prj{i}", name=f"prj{i}")
                       for i in range(2)]
                pob = [psm.tile([P, D + 1], fdt, tag=f"po{i}", name=f"po{i}")
                       for i in range(2)]

                # out-DMA ring rotation for the batched per-(h,c) stores
                orings = [nc.sync, nc.gpsimd, nc.scalar]
                ocnt = [0]

                units = []          # (key, cost, fn)
                emitted = set()

                def emit_one():
                    key, cost, fn = units.pop(0)
                    fn()
                    on_pe(cost)
                    emitted.add(key)

                def pad_until(target):
                    while clk["pe"] < target and units:
                        emit_one()

                def flush_until(key):
                    while key not in emitted and units:
                        emit_one()

                # --- unit builders ---
                def v_unit(j, e0):
                    def fn(j=j, e0=e0):
                        vb = vvb[j % 2]
                        for e in range(e0, e0 + 4):
                            nc.tensor.matmul(
                                vb[:], xts[e][:, P * j:P * (j + 1)],
                                wv_all[:, F * e:F * (e + 1)],
                                start=(e == 0), stop=(e == NE - 1))
                        if e0 == 12:
                            # fold the v bias in here: out of the softmax,
                            # sum a*(v+bv)/sum a == sum a*v/sum a + bv
                            for h in range(HPC):
                                nc.vector.tensor_add(
                                    vA[h][j][:, 0:D], vb[:, D * h:D * (h + 1)],
                                    bvb_sb[:, D * h:D * (h + 1)])
                            on_dve(VEVAC)
                    return (("v", j, e0), 4 * MM512, fn)

                def qk_unit(m, h, c, e0, mk=False):
                    def fn(m=m, h=h, c=c, e0=e0, mk=mk):
                        if mk and m == "q":
                            qT[h] = qkp.tile([P, S], bdt, tag="qT",
                                             name=f"qT{h}")
                        if mk and m == "k":
                            kT[h] = qkp.tile([P, S], bdt, tag="kT",
                                             name=f"kT{h}")
                        pt = prj[0] if m == "q" else prj[1]
                        wt = wq_all[h] if m == "q" else wk_all[h]
                        for e in range(e0, e0 + 4):
                            nc.tensor.matmul(
                                pt[:], wt[:, P * e:P * (e + 1)],
                                xts[e][:, 512 * c:512 * (c + 1)],
                                start=(e == 0), stop=(e == NE - 1))
                        if e0 == 12:
                            qk_evac(pt, h, c, m == "q")
                    return ((m, h, c, e0), 4 * MM512, fn)

                def queue_v(j):
                    for e0 in (0, 4, 8, 12):
                        units.append(v_unit(j, e0))

                def queue_qk(h):
                    load_wqk(h)
                    if h == 3:
                        # K first (scores(3,c) need all K chunks <= c), Q in
                        # descending-c order matching head-3's chunk order:
                        # leftover Q units double as tail PE pad work.
                        seq = [("k", c) for c in range(NCH)]
                        seq += [("q", c) for c in (3, 2, 1, 0)]
                    else:
                        seq = [(m, c) for c in range(NCH)
                               for m in ("q", "k")]
                    seen = set()
                    for m, c in seq:
                        for e0 in (0, 4, 8, 12):
                            units.append(
                                qk_unit(m, h, c, e0,
                                        mk=(m not in seen and e0 == 0)))
                        seen.add(m)

                # --- attention ---
                def attn(h, c, defer_pv=False, pool=None):
                    pool = pool or etp
                    nt = 4 * c + 4
                    if h == 3:
                        flush_until(("q", h, c, 12))
                        if c > 0:
                            # prefetch-flush the NEXT chunk's projections so
                            # their evac->swap->RoPE chain hides under this
                            # chunk's attention instead of stalling the PE
                            flush_until(("q", h, c - 1, 12))
                    elif h > 0:
                        flush_until(("k", h, c, 12))
                        if c < 3:
                            flush_until(("k", h, c + 1, 12))
                    if h > 0:
                        # pad until the (virtual) RoPE-chain completion of
                        # the chunk we are about to attend
                        pad_until(clk["dve"] - 600.0)
                    order = list(range(4 * c, nt)) + list(range(0, 4 * c))
                    sel = [None] * nt     # (tile, col_offset)
                    selm = [None] * nt    # masked diag block [P,128]
                    for t in order:
                        diag = t >= 4 * c
                        o = P * (t % 4) if diag else 0
                        w = 512 - o
                        cs = slice(512 * c + o, 512 * (c + 1))
                        ps_sc = scb[t % 2]
                        nc.tensor.matmul(ps_sc[:, 0:w],
                                         kT[h][:, P * t:P * (t + 1)],
                                         qT[h][:, cs], start=True, stop=True)
                        on_pe(_mm_cost(w))
                        tp = pool if pool is etdp else (
                            et3p if t >= 12 else etp)
                        et = tp.tile([P, w], bdt, tag=f"et{t}",
                                     name=f"et{h}_{c}_{t}")
                        nc.scalar.activation(et[:], ps_sc[:, 0:w], Exp,
                                             scale=SM_SCALE)
                        on_act(_exp_cost(w))
                        sel[t] = (et, o)
                        if diag:
                            etm = tp.tile([P, P], bdt, tag=f"etm{t % 4}",
                                          name=f"etm{h}_{c}_{t}")
                            nc.vector.tensor_mul(etm[:], et[:, 0:P],
                                                 mask_sb[:])
                            on_dve(MASK_DVE)
                            selm[t] = etm
                        pad_until(clk["act"] - 2 * _exp_cost(512))
                    gate = max(clk["act"], clk["dve"])
                    if defer_pv:
                        return sel, selm
                    if h == 0:
                        flush_until(("v", 4 * c + 3, 12))
                    pad_until(gate + MARGIN)
                    pv(h, c, sel, selm)

                def pv(h, c, sel, selm, as_units=False, split_out=False):
                    mk = []
                    ogc = []   # group staging tile, created lazily at emit

                    def do_j(jj, h, c, sel, selm):
                        if not ogc:
                            ogc.append(osp.tile([P, 4 * D], fdt, tag="og",
                                                name=f"og{h}_{c}", bufs=3))
                        og = ogc[0]
                        j = 4 * c + jj
                        po = pob[jj % len(pob)]
                        for t in range(j + 1):
                            if t == j:
                                src = selm[t][:]
                            else:
                                et, o = sel[t]
                                lo = P * jj - o
                                src = et[:, lo:lo + P]
                            nc.tensor.matmul(po[:], src, vA[h][t][:],
                                             start=(t == 0), stop=(t == j))
                        rec = osp.tile([P, 1], fdt, tag="rec",
                                       name=f"rec{h}_{j}")
                        nc.vector.reciprocal(rec[:], po[:, D:D + 1])
                        on_dve(PV_DVE)
                        # PV evac on the ACT engine: out = po * (1/den)
                        nc.scalar.activation(og[:, D * jj:D * (jj + 1)],
                                             po[:, 0:D], Ident, scale=rec[:])
                        on_act(PV_ACT)

                        def store(j0, nj):
                            r = orings[ocnt[0] % 3]
                            ocnt[0] += 1
                            r.dma_start(
                                outd[512 * c + P * j0:
                                     512 * c + P * (j0 + nj),
                                     D * h:D * (h + 1)].rearrange(
                                         "(j p) d -> p j d", p=P),
                                og[:, D * j0:D * (j0 + nj)].rearrange(
                                    "p (j d) -> p j d", j=nj))

                        if split_out:
                            # tail: write in j-pairs so the final HBM
                            # write-receipt starts as early as possible
                            if jj == 1:
                                store(0, 2)
                            elif jj == 3:
                                store(2, 2)
                        elif jj == 3:
                            store(0, 4)

                    for jj in range(4):
                        def fn(jj=jj, h=h, c=c, sel=sel, selm=selm):
                            do_j(jj, h, c, sel, selm)
                        cost = (4 * c + jj + 1) * MMPV
                        if as_units:
                            mk.append((("pvd", h, c, jj), cost, fn))
                        else:
                            fn()
                            on_pe(cost)
                    return mk

                # ---------------- the schedule ----------------
                for j in range(NSQ):
                    queue_v(j)
                queue_qk(1)
                # bridge the head-0 RoPE-chain latency with V work so the
                # PE isn't head-of-line blocked on scores(0,0)'s inputs
                flush_until(("v", 1, 12))
                for c in range(NCH):
                    attn(0, c)
                # head-0 attention done: V psum banks are free; deepen the
                # PV rotation to 4 banks for heads 1-3
                pob.append(psm.tile([P, D + 1], fdt, tag="vv0", name="po2"))
                pob.append(psm.tile([P, D + 1], fdt, tag="vv1", name="po3"))
                queue_qk(2)
                d1 = None
                for c in range(NCH):
                    if c == 3:
                        d1 = attn(1, c, defer_pv=True, pool=etdp2)
                    else:
                        attn(1, c)
                queue_qk(3)
                dsel = dselm = None
                for c in range(NCH):
                    if c == 3:
                        dsel, dselm = attn(2, c, defer_pv=True, pool=etdp)
                    else:
                        attn(2, c)
                # head 3 descending, deferred PV(1,3)+PV(2,3) as tail pads
                units.extend(pv(1, 3, d1[0], d1[1], as_units=True))
                units.extend(pv(2, 3, dsel, dselm, as_units=True))
                g3 = {}
                for c in (3, 2, 1, 0):
                    s, m = attn(3, c, defer_pv=True)
                    g3[c] = (s, m, max(clk["act"], clk["dve"]))
                    if c == 3:
                        continue
                    sd, md, gate = g3[c + 1]
                    if c == 0:
                        # spend reserve pads now: the kernel must end on
                        # dependency-free matmuls, not an exp wait
                        while len(units) > 2:
                            emit_one()
                    pad_until(gate + MARGIN)
                    pv(3, c + 1, sd, md, split_out=(c == 0))
                sd, md, gate = g3[0]
                while units:
                    emit_one()
                pv(3, 0, sd, md, split_out=True)

    nc.compile()
    return nc


def get_compiled():
    global _compiled
    if _compiled is None:
        _compiled = _build()
    return _compiled


def expected_slice_core0(expected):
    return expected[0, :, 0:F]


def make_in_maps(logits, Wq, bq, Wk, bk, Wv, bv):
    cosf, ssin = _rope_tables()
    maskm = _mask_tile()
    xts = [np.ascontiguousarray(np.asarray(logits)[b].T).astype(BF16)
           for b in range(B)]

    def permW(Wm, rows):
        Wp = np.asarray(Wm)[rows].reshape(HPC, D, E)[:, _PERM, :].reshape(F, E)
        WT = Wp.T  # [E, F]
        # pre-arrange to SBUF layout: out[p, h*S + e*P + f] = WT[e*P+p, h*P+f]
        A = WT.reshape(NE, P, HPC, P).transpose(1, 2, 0, 3)
        return np.ascontiguousarray(A.reshape(P, HPC * NE * P)).astype(BF16)

    def permWv(Wm, rows):
        WT = np.asarray(Wm)[rows].T  # [E, F]
        # out[p, e*F + f] = WT[e*P+p, f]
        A = WT.reshape(NE, P, F).transpose(1, 0, 2)
        return np.ascontiguousarray(A.reshape(P, NE * F)).astype(BF16)

    def permb(bvec, rows):
        return np.ascontiguousarray(
            np.asarray(bvec)[rows].reshape(HPC, D)[:, _PERM].T
        ).astype(np.float32)

    in_maps = []
    for core in range(NCORES):
        b, g = divmod(core, 4)
        rows = slice(F * g, F * (g + 1))
        in_maps.append({
            "xt": xts[b],
            "wqt": permW(Wq, rows),
            "wkt": permW(Wk, rows),
            "wvt": permWv(Wv, rows),
            "bqd": permb(bq, rows),
            "bkd": permb(bk, rows),
            "bvbd": np.ascontiguousarray(np.broadcast_to(
                np.asarray(bv)[rows].astype(np.float32), (P, F))),
            "cosd": cosf,
            "ssind": ssin,
            "maskd": maskm,
        })
    return in_maps


def kernel(logits, Wq, bq, Wk, bk, Wv, bv, **_ignored):
    global LAST_RESULT
    from concourse.bass_utils import run_bass_kernel_spmd

    nc = get_compiled()
    in_maps = make_in_maps(logits, Wq, bq, Wk, bk, Wv, bv)
    res = run_bass_kernel_spmd(nc, in_maps, list(range(NCORES)))
    LAST_RESULT = res
    out = np.empty((B, S, H * D), dtype=np.float32)
    for core in range(NCORES):
        b, g = divmod(core, 4)
        out[b, :, F * g:F * (g + 1)] = res.results[core]["out"]
    return out
